# revision 15
# baseline (speedup 1.0000x reference)
"""Trainium2 Bass kernel for GNN message passing (nn_BDLModule_34488587387542).

Computation (N=100000 nodes, E=1600000 edges, DIM=128):
    deg  = out-degree(src);  a = rsqrt(deg)
    h0   = per-node block rotation of x (8 bundles of 4x4)
    h2   = S S h0,  S = diag(a) A^T diag(a)   (2 propagation steps)
    h3   = inverse rotation of h2
    out  = GELU_exact(h3 @ w1.T + b1) @ w2.T + b2

Sharding: nodes partitioned contiguously across 8 cores (12500 each, padded
to 12544 = 98 tiles); fp16 node tables replicated between steps with one
AllGather per step. Edges live on the core owning dst. Per propagation
step: dma_gather h[src] rows from the replicated table (4 int16-addressable
chunks, 4-way queue-split), build one-hot dst matrices in batched DVE ops
(is_equal vs iota with stride-0 broadcast APs), scatter via PE matmuls into
bank-packed PSUM accumulators (4+3 dst tiles per 2 banks -- one start=True
per bank since start clears the whole bank's has_written bits). Edge groups
are packed densely per (supertile, chunk) bucket without per-tile 128-
alignment: a group spanning a tile boundary emits one matmul per touched
tile with its own one-hot column (union of incidences over cores keeps the
program SPMD). Norm coefs are folded into the stored tables (a on R1 input,
a^2 between steps, a before the inverse rotation). The P2 emit is stage-
major per supertile: batched PSUM scale-copies, interleaved DVE/Pool
rotation chains (3 DVE / 4 Pool to keep DVE under the one-hot load), PE
transposes into one fp16 PSUM bank, and an fp16-weight FFN in tile pairs
(GELU + final bias on the otherwise-idle Act engine).
"""
import os
import sys

sys.path.append("/opt/trn_rl_repo")

import numpy as np

N_NODES = 100000
N_EDGES = 1600000
DIM = 128
HID = 256
N_CORES = 8
NSH = 12500
NSHP = 12544
NT = NSHP // 128            # 98
NREP = NSHP * N_CORES
CHUNK = NREP // 4           # 25088
N_CHUNKS = 4
PAD_DST = 1000.0
TS = 7
NS = NT // TS               # 14

LAST_RESULTS = None
LAST_NC = None
LAST_IN_MAPS = None


def _wrap_idx(idx_flat: np.ndarray) -> np.ndarray:
    w = idx_flat.reshape(-1, 16).T.astype(np.int16)
    return np.tile(w, (8, 1))


def preprocess(x, node_rep, src, dst, w1, b1, w2, b2):
    deg = np.bincount(src, minlength=N_NODES).astype(np.float64)
    a = (1.0 / np.sqrt(deg)).astype(np.float32)
    a2 = (1.0 / deg).astype(np.float32)

    src_rrow = (src // NSH) * NSHP + (src % NSH)
    dst_core = dst // NSH

    # bucket = (supertile, chunk); edges sorted by tile within the bucket
    n_buckets = NS * N_CHUNKS
    per_core = []
    counts = np.zeros((N_CORES, n_buckets), np.int64)
    tile_counts = np.zeros((N_CORES, n_buckets, TS), np.int64)
    for c in range(N_CORES):
        m = dst_core == c
        dl = (dst[m] - c * NSH).astype(np.int64)
        sr = src_rrow[m]
        tile_id = dl // 128
        chunk_id = sr // CHUNK
        s_id = tile_id // TS
        key = (s_id * N_CHUNKS + chunk_id) * TS + tile_id % TS
        order = np.argsort(key, kind="stable")
        per_core.append((dl[order], sr[order], key[order] // TS))
        kb = key // TS
        counts[c] = np.bincount(kb, minlength=n_buckets)
        for u in range(TS):
            tile_counts[c, :, u] = np.bincount(
                kb[key % TS == u], minlength=n_buckets)

    # groups per bucket = ceil(max-over-cores edges / 128)
    G = np.maximum(np.ceil(counts.max(axis=0) / 128.0).astype(np.int64), 1)
    g_off = np.concatenate([[0], np.cumsum(G)])
    total_groups = int(g_off[-1])
    total_edges_padded = total_groups * 128

    # union (group, tile) incidence over cores; per-core group boundaries
    # fall at cumulative tile counts within the bucket
    inc = [set() for _ in range(n_buckets)]
    for b in range(n_buckets):
        for c in range(N_CORES):
            cum = 0
            for u in range(TS):
                n_u = int(tile_counts[c, b, u])
                if n_u == 0:
                    continue
                glo = cum // 128
                ghi = (cum + n_u - 1) // 128
                for g in range(glo, ghi + 1):
                    inc[b].add((g, u))
                cum += n_u
        if not inc[b]:
            inc[b].add((0, 0))
    # ensure every tile has at least one incidence somewhere (zero-init)
    has_tile = np.zeros(NT, bool)
    for b in range(n_buckets):
        s = b // N_CHUNKS
        for (g, u) in inc[b]:
            has_tile[s * TS + u] = True
    for t in range(NT):
        if not has_tile[t]:
            inc[(t // TS) * N_CHUNKS].add((0, t % TS))

    # column layout: contiguous per bucket, ordered (g, u)
    mm_lists = []                     # per bucket: [(g, u, col)]
    col_off = np.zeros(n_buckets + 1, np.int64)
    ncols = 0
    for b in range(n_buckets):
        col_off[b] = ncols
        lst = []
        for (g, u) in sorted(inc[b]):
            lst.append((g, u, ncols))
            ncols += 1
        mm_lists.append(lst)
    col_off[n_buckets] = ncols

    structure = {"G": G, "g_off": g_off, "mm_lists": mm_lists,
                 "col_off": col_off, "ncols": ncols}

    iota = np.tile(np.arange(128, dtype=np.float16), (128, 1))
    ident = np.eye(128, dtype=np.float16)
    w1t = np.ascontiguousarray(w1.T).astype(np.float16)
    b1h = np.ascontiguousarray(b1.reshape(2, 128).T)
    w2t2 = np.ascontiguousarray(
        w2.T.reshape(2, 128, DIM).transpose(1, 0, 2)).astype(np.float16)
    b2c = np.ascontiguousarray(b2.reshape(128, 1))

    # dense (bucket, group, tile) -> column lookup for vectorized fill
    col_arr = np.full((n_buckets, int(G.max()), TS), -1, np.int64)
    for b in range(n_buckets):
        for (g, u, col) in mm_lists[b]:
            col_arr[b, g, u] = col

    in_maps = []
    for c in range(N_CORES):
        dl, sr, kb = per_core[c]
        n = dl.shape[0]
        within = np.arange(n) - np.concatenate(
            [[0], np.cumsum(np.bincount(kb, minlength=n_buckets))])[kb]
        pos = g_off[kb] * 128 + within
        idx_pad = np.zeros(total_edges_padded, np.int64)
        idx_pad[pos] = sr % CHUNK
        # dst goes into the edge's (group, tile) column at its group slot
        dst_cols = np.full((ncols, 128), PAD_DST, np.float16)
        grp = within // 128
        slot = within % 128
        tile_u = (dl // 128) % TS
        cols = col_arr[kb, grp, tile_u]
        assert (cols >= 0).all()
        dst_cols[cols, slot] = (dl % 128).astype(np.float16)

        rows = slice(c * NSH, (c + 1) * NSH)
        x_sh = np.zeros((NSHP, DIM), np.float32)
        x_sh[:NSH] = x[rows]
        rep_sh = np.zeros((NSHP, DIM), np.float32)
        rep_sh[:NSH] = node_rep[rows].reshape(NSH, DIM)
        a_pad = np.zeros(NSHP, np.float32)
        a_pad[:NSH] = a[rows]
        a2_pad = np.zeros(NSHP, np.float32)
        a2_pad[:NSH] = a2[rows]

        in_maps.append({
            "x_sh": x_sh,
            "rep_sh": rep_sh,
            "idx_all": _wrap_idx(idx_pad),
            "dst_all": np.ascontiguousarray(dst_cols.T),   # [128, ncols]
            "a_col": np.ascontiguousarray(a_pad.reshape(NT, 128).T),
            "a2_col": np.ascontiguousarray(a2_pad.reshape(NT, 128).T),
            "iota": iota,
            "ident": ident,
            "w1t": w1t,
            "b1h": b1h,
            "w2t2": w2t2,
            "b2c": b2c,
        })
    return in_maps, structure, total_groups


def build_nc(structure, total_groups, single_core_timing=False, ablate=(),
             n_queues=4, gather_split=4, single_packet=False,
             stop_after=None):
    import concourse.bacc as bacc
    import concourse.mybir as mybir
    import concourse.tile as tile

    f32 = mybir.dt.float32
    f16 = mybir.dt.float16
    nc = bacc.Bacc("TRN2", target_bir_lowering=False, debug=False,
                   num_devices=1 if single_core_timing else N_CORES,
                   num_swdge_queues=n_queues)

    G = structure["G"]
    g_off = structure["g_off"]
    mm_lists = structure["mm_lists"]
    ncols = structure["ncols"]
    gsk_max = int(G.max())
    ncol_max = max(len(l) for l in mm_lists)

    x_sh = nc.dram_tensor("x_sh", [NSHP, DIM], f32, kind="ExternalInput")
    rep_sh = nc.dram_tensor("rep_sh", [NSHP, DIM], f32, kind="ExternalInput")
    idx_all = nc.dram_tensor("idx_all", [128, total_groups * 8],
                             mybir.dt.int16, kind="ExternalInput")
    dst_all = nc.dram_tensor("dst_all", [128, ncols], f16,
                             kind="ExternalInput")
    a_col = nc.dram_tensor("a_col", [128, NT], f32, kind="ExternalInput")
    a2_col = nc.dram_tensor("a2_col", [128, NT], f32, kind="ExternalInput")
    iota = nc.dram_tensor("iota", [128, 128], f16, kind="ExternalInput")
    ident = nc.dram_tensor("ident", [128, 128], f16, kind="ExternalInput")
    w1t = nc.dram_tensor("w1t", [DIM, HID], f16, kind="ExternalInput")
    b1h = nc.dram_tensor("b1h", [128, 2], f32, kind="ExternalInput")
    w2t2 = nc.dram_tensor("w2t2", [128, 2, DIM], f16, kind="ExternalInput")
    b2c = nc.dram_tensor("b2c", [128, 1], f32, kind="ExternalInput")
    out_t = nc.dram_tensor("out_t", [DIM, NSHP], f32, kind="ExternalOutput")

    with tile.TileContext(nc) as tc:
        with (
            tc.tile_pool(name="const", bufs=1) as cp,
            tc.tile_pool(name="io", bufs=2) as iop,
            tc.tile_pool(name="rotp", bufs=2) as rotp,
            tc.tile_pool(name="gath", bufs=6) as gp,
            tc.tile_pool(name="m2", bufs=3) as m2p,
            tc.tile_pool(name="outp", bufs=2) as op,
            tc.tile_pool(name="acc", bufs=2, space="PSUM") as accp,
            tc.tile_pool(name="work", bufs=2, space="PSUM") as wp,
            tc.tile_pool(name="dram", bufs=1, space="DRAM") as dp,
        ):
            iota_sb = cp.tile([128, 128], f16)
            nc.sync.dma_start(iota_sb[:], iota[:])
            id_sb = cp.tile([128, 128], f16)
            nc.sync.dma_start(id_sb[:], ident[:])
            idx_sb = cp.tile([128, total_groups * 8], mybir.dt.int16)
            nc.sync.dma_start(idx_sb[:], idx_all[:])
            dst_sb = cp.tile([128, ncols], f16)
            nc.sync.dma_start(dst_sb[:], dst_all[:])
            a_sb = cp.tile([128, NT], f32)
            nc.sync.dma_start(a_sb[:], a_col[:])
            a2_sb = cp.tile([128, NT], f32)
            nc.sync.dma_start(a2_sb[:], a2_col[:])
            w1t_sb = cp.tile([DIM, HID], f16)
            nc.sync.dma_start(w1t_sb[:], w1t[:])
            b1h_sb = cp.tile([128, 2], f32)
            nc.sync.dma_start(b1h_sb[:], b1h[:])
            w2t2_sb = cp.tile([128, 2, DIM], f16)
            nc.sync.dma_start(w2t2_sb[:], w2t2[:])
            b2c_sb = cp.tile([128, 1], f32)
            nc.sync.dma_start(b2c_sb[:], b2c[:])

            rep_space = "Local" if single_core_timing else "Shared"
            g0_sh = dp.tile([NSHP, DIM], f16)
            g0_rep = dp.tile([NREP, DIM], f16, addr_space=rep_space)
            g1_sh = dp.tile([NSHP, DIM], f16)
            g1_rep = dp.tile([NREP, DIM], f16, addr_space=rep_space)

            def allgather(sh, rep):
                if single_core_timing:
                    nc.sync.dma_start(rep[0:NSHP, :], sh[:])
                else:
                    nc.gpsimd.collective_compute(
                        "AllGather", mybir.AluOpType.bypass,
                        ins=[sh.opt()], outs=[rep.opt()],
                        replica_groups=[list(range(N_CORES))],
                    )

            def bcast_col(col_ap, n):
                return col_ap.unsqueeze(2).broadcast_to((128, n, 128))

            def rotations(xb, rb, dest, tmp, transposed):
                for d in range(4):
                    for u in range(TS):
                        eng = nc.vector if u in (0, 2, 4) else nc.gpsimd
                        x4 = xb[:, u, :].rearrange("p (b d e) -> p b d e",
                                                   b=8, d=4, e=4)
                        r4 = rb[:, u, :].rearrange("p (b c d) -> p b c d",
                                                   b=8, c=4, d=4)
                        if transposed:
                            a_d = r4[:, :, d, :].unsqueeze(3)
                        else:
                            a_d = r4[:, :, :, d].unsqueeze(3)
                        a_d = a_d.broadcast_to((128, 8, 4, 4))
                        b_d = x4[:, :, d, :].unsqueeze(2).broadcast_to(
                            (128, 8, 4, 4))
                        dst4 = (dest if d == 0 else tmp)[:, u, :].rearrange(
                            "p (b c e) -> p b c e", b=8, c=4, e=4)
                        eng.tensor_tensor(dst4, a_d, b_d,
                                          op=mybir.AluOpType.mult)
                        if d > 0:
                            eng.tensor_tensor(dest[:, u, :], dest[:, u, :],
                                              tmp[:, u, :],
                                              op=mybir.AluOpType.add)

            def sh_rows(dram, s):
                return dram[s * TS * 128:(s + 1) * TS * 128, :].rearrange(
                    "(q p) d -> p q d", p=128)

            for s in range(NS):
                xp = iop.tile([128, TS, DIM], f32, tag="xp")
                nc.sync.dma_start(xp[:], sh_rows(x_sh, s))
                rp = iop.tile([128, TS, DIM], f32, tag="rp")
                nc.sync.dma_start(rp[:], sh_rows(rep_sh, s))
                xs = rotp.tile([128, TS, DIM], f32, tag="xs")
                nc.vector.tensor_tensor(
                    xs[:], xp[:], bcast_col(a_sb[:, s * TS:(s + 1) * TS], TS),
                    op=mybir.AluOpType.mult)
                g0p = op.tile([128, TS, DIM], f16, tag="g0p")
                tmp = rotp.tile([128, TS, DIM], f16, tag="tmpr")
                rotations(xs, rp, g0p, tmp, transposed=False)
                nc.sync.dma_start(sh_rows(g0_sh, s), g0p[:])

            allgather(g0_sh, g0_rep)

            def early_out(dram):
                fin = op.tile([128, 128], f32, tag="fin")
                nc.gpsimd.dma_start(fin[:], dram[0:128, :])
                nc.sync.dma_start(out_t[:, 0:128], fin[:])

            if stop_after == "r1":
                early_out(g0_rep)

            def prop_step(g_rep, emit_cb):
                for s in range(NS):
                    acc0 = accp.tile([128, 4, 128], f32, tag="acc0")
                    acc1 = accp.tile([128, 3, 128], f32, tag="acc1")
                    bank_started = [False, False]
                    bank_left = [0, 0]
                    for k in range(N_CHUNKS):
                        for (g, u, col) in mm_lists[s * N_CHUNKS + k]:
                            bank_left[0 if u < 4 else 1] += 1
                    for k in range(N_CHUNKS):
                        b_id = s * N_CHUNKS + k
                        gc = int(G[b_id])
                        c0g = int(g_off[b_id])
                        cols = mm_lists[b_id]
                        ncol = len(cols)
                        col0 = cols[0][2]
                        gth = gp.tile([128, gsk_max, DIM], f16, tag="gth")
                        if "gather" not in ablate:
                            bounds = [gc * i // gather_split
                                      for i in range(gather_split + 1)]
                            for h in range(gather_split):
                                lo, hi = bounds[h], bounds[h + 1]
                                if lo == hi:
                                    continue
                                nc.gpsimd.dma_gather(
                                    gth[:, lo:hi, :],
                                    g_rep[k * CHUNK:(k + 1) * CHUNK, :],
                                    idx_sb[:, (c0g + lo) * 8:(c0g + hi) * 8],
                                    128 * (hi - lo), 128 * (hi - lo), DIM,
                                    single_packet=single_packet,
                                    queue_num=(gather_split * k + h)
                                    % n_queues,
                                )
                        else:
                            nc.sync.dma_start(gth[:, 0, :], g_rep[0:128, :])
                        m2 = m2p.tile([128, ncol_max, 128], f16, tag="m2")
                        if "onehot" not in ablate:
                            nc.vector.tensor_tensor(
                                m2[:, 0:ncol, :],
                                iota_sb[:].unsqueeze(1).broadcast_to(
                                    (128, ncol, 128)),
                                bcast_col(dst_sb[:, col0:col0 + ncol], ncol),
                                op=mybir.AluOpType.is_equal,
                            )
                        else:
                            nc.vector.tensor_scalar(
                                m2[:, 0, 0:8], iota_sb[:, 0:8], 0.0,
                                None, op0=mybir.AluOpType.mult)
                        for (g, u, col) in cols:
                            b = 0 if u < 4 else 1
                            acc = acc0 if b == 0 else acc1
                            if "mm" in ablate and bank_started[b]:
                                bank_left[b] -= 1
                                continue
                            bank_left[b] -= 1
                            nc.tensor.matmul(
                                acc[:, u % 4, :],
                                m2[:, col - col0, :] if "onehot" not in
                                ablate else m2[:, 0, :],
                                gth[:, g if "gather" not in ablate else 0, :],
                                start=not bank_started[b],
                                stop=("mm" in ablate) or (bank_left[b] == 0),
                                skip_group_check=True,
                            )
                            bank_started[b] = True
                    emit_cb(s, acc0, acc1)

            def p1_emit(s, acc0, acc1):
                g1p = op.tile([128, TS, DIM], f16, tag="g1p")
                nc.vector.tensor_tensor(
                    g1p[:, 0:4, :], acc0[:],
                    bcast_col(a2_sb[:, s * TS:s * TS + 4], 4),
                    op=mybir.AluOpType.mult)
                nc.vector.tensor_tensor(
                    g1p[:, 4:7, :], acc1[:],
                    bcast_col(a2_sb[:, s * TS + 4:s * TS + 7], 3),
                    op=mybir.AluOpType.mult)
                nc.sync.dma_start(sh_rows(g1_sh, s), g1p[:])

            if stop_after != "r1":
                prop_step(g0_rep, p1_emit)
                allgather(g1_sh, g1_rep)

            if stop_after == "p1":
                early_out(g1_rep)

            def p2_emit(s, acc0, acc1):
                rp2 = iop.tile([128, TS, DIM], f32, tag="rp2")
                nc.sync.dma_start(rp2[:], sh_rows(rep_sh, s))
                h2b = rotp.tile([128, TS, DIM], f32, tag="h2b")
                nc.vector.tensor_tensor(
                    h2b[:, 0:4, :], acc0[:],
                    bcast_col(a_sb[:, s * TS:s * TS + 4], 4),
                    op=mybir.AluOpType.mult)
                nc.vector.tensor_tensor(
                    h2b[:, 4:7, :], acc1[:],
                    bcast_col(a_sb[:, s * TS + 4:s * TS + 7], 3),
                    op=mybir.AluOpType.mult)
                h3b = rotp.tile([128, TS, DIM], f16, tag="h3b")
                tmp = rotp.tile([128, TS, DIM], f16, tag="tmp2")
                rotations(h2b, rp2, h3b, tmp, transposed=True)
                bt = op.tile([128, TS, DIM], f32, tag="op2")
                tp = wp.tile([128, 8, 128], f16, tag="tp")
                for i in range(TS):
                    nc.tensor.transpose(tp[:, i, :], h3b[:, i, :], id_sb[:])
                h3t = rotp.tile([128, TS, 128], f16, tag="h3t")
                nc.scalar.copy(h3t[:], tp[:, 0:TS, :])
                for p0 in range(0, TS, 2):
                    pn = min(2, TS - p0)
                    ps1 = wp.tile([128, 4, 128], f32, tag="ps1")
                    for i in range(pn):
                        for h in range(2):
                            nc.tensor.matmul(
                                ps1[:, 2 * h + i, :],
                                w1t_sb[:, h * 128:(h + 1) * 128],
                                h3t[:, p0 + i, :],
                                start=(i == 0 and h == 0), stop=True,
                                skip_group_check=True)
                    act = rotp.tile([128, 4, 128], f16, tag="act")
                    for h in range(2):
                        nc.scalar.activation(
                            act[:, 2 * h:2 * h + pn, :],
                            ps1[:, 2 * h:2 * h + pn, :],
                            mybir.ActivationFunctionType.Gelu,
                            bias=b1h_sb[:, h:h + 1])
                    for i in range(pn):
                        for h in range(2):
                            nc.tensor.matmul(
                                ps1[:, i, :], w2t2_sb[:, h, :],
                                act[:, 2 * h + i, :],
                                start=(i == 0 and h == 0), stop=(h == 1),
                                skip_group_check=True)
                    nc.scalar.activation(
                        bt[:, p0:p0 + pn, :], ps1[:, 0:pn, :],
                        mybir.ActivationFunctionType.Identity,
                        bias=b2c_sb[:])
                nc.sync.dma_start(
                    out_t[:, s * TS * 128:(s + 1) * TS * 128],
                    bt[:].rearrange("p q d -> p (q d)"))

            if stop_after is None:
                prop_step(g1_rep, p2_emit)

    nc.compile()
    return nc


def kernel(x, node_rep, src, dst, w1, b1, w2, b2):
    global LAST_RESULTS, LAST_NC, LAST_IN_MAPS
    from concourse import bass_utils

    x = np.asarray(x, np.float32)
    node_rep = np.asarray(node_rep, np.float32)
    src = np.asarray(src, np.int64)
    dst = np.asarray(dst, np.int64)
    w1 = np.asarray(w1, np.float32)
    b1 = np.asarray(b1, np.float32)
    w2 = np.asarray(w2, np.float32)
    b2 = np.asarray(b2, np.float32)

    in_maps, structure, total_groups = preprocess(
        x, node_rep, src, dst, w1, b1, w2, b2)
    nc = build_nc(structure, total_groups)
    res = bass_utils.run_bass_kernel_spmd(
        nc, in_maps, core_ids=list(range(N_CORES)),
    )
    LAST_RESULTS = res
    LAST_NC = nc
    LAST_IN_MAPS = in_maps
    out = np.concatenate(
        [res.results[c]["out_t"].T[:NSH] for c in range(N_CORES)], axis=0)
    return np.ascontiguousarray(out)
